# revision 5
# baseline (speedup 1.0000x reference)
"""Trainium2 Bass kernel for nn_DiffusionModel (reverse-diffusion sampling).

Strategy (data-parallel, per sharding hint):
  - 8 cores, batch 128 -> 16 rows per core; no cross-core communication.
  - 1000 serial steps; per step a 4-layer 512x512 MLP + elementwise update.
  - Weights live in SBUF in fp16 (PE fast-weight-load eligible), stationary
    operand of every matmul; activations kept feature-on-partition ("xT"
    layout, [128 partitions, 4 chunks x 16 batch cols]) so no transposes are
    ever needed: each layer's PSUM output is already next layer's input layout.
  - Time embedding is a per-step scalar -> folded into a per-step layer-1 bias
    table on the host (like betas/alphas_cumprod schedule).
  - Reverse-diffusion update x' = A_t*x + B_t*noise_pred + C_t*noise with
    host-precomputed A/B/C; C_t*noise (+ B_t*b4 fold) pre-scaled on host.
  - fp32 x master + fp32 PSUM accumulation; only the matmul datapath is fp16
    (validated: ~3e-5 relative error vs fp32 reference over 1000 steps).
"""

import os
import numpy as np

T = 1000
B = 128
D = 512
H = 512
BETA_START = 0.0001
BETA_END = 0.02
NCORES = 8
BSH = B // NCORES        # 16 batch rows per core
NCH = D // 128           # 4 feature chunks
COLS = NCH * BSH         # 64 sbuf cols per activation tile
UNROLL = 8

_nc_cache = {}
LAST_RESULTS = None


# ---------------------------------------------------------------- host tables
def host_tables(w1, b1, tw1, tb1, tw2, tb2, b4, nsteps=T):
    """Per-step (s = 0..T-1, t = T-1-s) coefficients and layer-1 bias table.

    x_{s+1} = A[s]*x + B[s]*noise_pred + C[s]*noise[t]
    bias1[s] = b1 + t_emb(t) * w1[512, :]
    """
    betas = np.linspace(BETA_START, BETA_END, T, dtype=np.float32)
    alphas = (1.0 - betas).astype(np.float32)
    ac = np.cumprod(alphas, dtype=np.float32)

    ts_rev = np.arange(T - 1, -1, -1)  # t for step s
    ac_t = ac[ts_rev].astype(np.float64)
    ac_prev = np.where(ts_rev > 0, ac[np.maximum(ts_rev - 1, 0)], 1.0).astype(np.float64)
    A = np.sqrt(ac_prev) / np.sqrt(ac_t)
    Bc = np.sqrt(1.0 - ac_prev) - A * np.sqrt(1.0 - ac_t)
    C = np.where(ts_rev > 0, np.sqrt(betas[ts_rev].astype(np.float64)), 0.0)

    # time-embedding scalar per t (same for every batch row)
    tnorm = (ts_rev.astype(np.float64) / T)[:, None]                  # [T,1]
    temb = np.maximum(tnorm @ tw1.astype(np.float64) + tb1.astype(np.float64), 0.0)
    temb = temb @ tw2.astype(np.float64) + tb2.astype(np.float64)     # [T,1]
    bias1 = b1.astype(np.float64)[None, :] + temb * w1[D].astype(np.float64)[None, :]

    A = A.astype(np.float32)[:nsteps]
    Bc = Bc.astype(np.float32)[:nsteps]
    C = C.astype(np.float32)[:nsteps]
    bias1 = bias1.astype(np.float32)[:nsteps]
    return A, Bc, C, bias1, ts_rev[:nsteps]


def pack_weights(w1, w2, w3, w4):
    """[128, 4*16*128] fp16: layer l chunk (k,m) at cols ((l*16)+(k*4+m))*128."""
    out = []
    for w in (w1[:D], w2, w3, w4):
        r = w.reshape(NCH, 128, NCH, 128).transpose(1, 0, 2, 3).reshape(128, NCH * NCH * 128)
        out.append(r)
    return np.ascontiguousarray(np.concatenate(out, axis=1)).astype(np.float16)


def to_dev_layout(x):
    """[16, 512] -> [128, 64] with dev[p, c*16+b] = x[b, c*128+p]."""
    return np.ascontiguousarray(
        x.reshape(BSH, NCH, 128).transpose(2, 1, 0).reshape(128, COLS)
    )


def from_dev_layout(xd):
    """[128, 64] -> [16, 512]."""
    return np.ascontiguousarray(
        xd.reshape(128, NCH, BSH).transpose(2, 1, 0).reshape(BSH, D)
    )


# ---------------------------------------------------------------- bass kernel
def build_nc(nsteps=T, unroll=UNROLL):
    import concourse.bass as bass
    import concourse.mybir as mybir
    import concourse.tile as tile
    from concourse import bacc
    from concourse.bass import ds

    f32 = mybir.dt.float32
    f16 = mybir.dt.float16
    Relu = mybir.ActivationFunctionType.Relu
    add = mybir.AluOpType.add
    mult = mybir.AluOpType.mult
    amax = mybir.AluOpType.max

    assert nsteps % unroll == 0
    niter = nsteps // unroll

    nc = bacc.Bacc("TRN2", target_bir_lowering=False)

    wts_d = nc.dram_tensor("wts", [128, 4 * 16 * 128], f16, kind="ExternalInput")
    bias1_d = nc.dram_tensor("bias1", [128, 4 * nsteps], f32, kind="ExternalInput")
    b23_d = nc.dram_tensor("b23", [128, 8], f32, kind="ExternalInput")
    cA_d = nc.dram_tensor("cA", [128, nsteps], f32, kind="ExternalInput")
    cB_d = nc.dram_tensor("cB", [128, nsteps], f32, kind="ExternalInput")
    noise_d = nc.dram_tensor("noise", [nsteps * 128, COLS], f32, kind="ExternalInput")
    xinit_d = nc.dram_tensor("xinit", [128, COLS], f32, kind="ExternalInput")
    xout_d = nc.dram_tensor("xout", [128, COLS], f32, kind="ExternalOutput")

    with tile.TileContext(nc) as tc:
        with (
            tc.tile_pool(name="const", bufs=1) as cpool,
            tc.tile_pool(name="noise", bufs=unroll) as npool,
            tc.tile_pool(name="acts", bufs=2) as hpool,
            tc.tile_pool(name="ps", bufs=1, space="PSUM") as pspool,
        ):
            wts = cpool.tile([128, 4 * 16 * 128], f16, tag="wts")
            bias1 = cpool.tile([128, 4 * nsteps], f32, tag="bias1")
            b23 = cpool.tile([128, 8], f32, tag="b23")
            cA = cpool.tile([128, nsteps], f32, tag="cA")
            cB = cpool.tile([128, nsteps], f32, tag="cB")
            x32a = cpool.tile([128, COLS], f32, tag="x32a")
            x32b = cpool.tile([128, COLS], f32, tag="x32b")
            x16a = cpool.tile([128, COLS], f16, tag="x16a")
            x16b = cpool.tile([128, COLS], f16, tag="x16b")

            nc.sync.dma_start(wts[:], wts_d[:])
            nc.sync.dma_start(bias1[:], bias1_d[:])
            nc.sync.dma_start(b23[:], b23_d[:])
            nc.sync.dma_start(cA[:], cA_d[:])
            nc.sync.dma_start(cB[:], cB_d[:])
            nc.sync.dma_start(x32a[:], xinit_d[:])
            nc.vector.tensor_copy(x16a[:], x32a[:])

            def wchunk(l, k, m):
                c0 = (l * 16 + k * 4 + m) * 128
                return wts[:, c0:c0 + 128]

            with tc.For_i(0, nsteps, unroll,
                          hint_engines=(mybir.EngineType.PE,)) as it:
                ntiles = []
                for u in range(unroll):
                    nt = npool.tile([128, COLS], f32, tag="noise", name=f"nt{u}")
                    nc.sync.dma_start(nt[:], noise_d[ds(it * 128 + u * 128, 128), :])
                    ntiles.append(nt)

                for u in range(unroll):
                    if u % 2 == 0:
                        x32c, x16c, x32n, x16n = x32a, x16a, x32b, x16b
                    else:
                        x32c, x16c, x32n, x16n = x32b, x16b, x32a, x16a

                    act = x16c
                    for l in range(3):
                        pms = [pspool.tile([128, BSH], f32, tag=f"pm{m}",
                                           name=f"pm{u}_{l}_{m}") for m in range(4)]
                        for m in range(4):
                            for k in range(4):
                                nc.tensor.matmul(
                                    pms[m][:, :], lhsT=wchunk(l, k, m),
                                    rhs=act[:, k * BSH:(k + 1) * BSH],
                                    start=(k == 0), stop=(k == 3))
                        h = hpool.tile([128, COLS], f16, tag=f"h{l}", name=f"h{u}_{l}")
                        for c in range(4):
                            if l == 0:
                                nc.vector.tensor_scalar(
                                    out=h[:, c * BSH:(c + 1) * BSH], in0=pms[c][:, :],
                                    scalar1=bias1[:, ds(it * 4 + (u * 4 + c), 1)],
                                    scalar2=0.0, op0=add, op1=amax)
                            else:
                                nc.scalar.activation(
                                    h[:, c * BSH:(c + 1) * BSH], pms[c][:, :], Relu,
                                    bias=b23[:, (l - 1) * 4 + c:(l - 1) * 4 + c + 1])
                        act = h

                    p4 = pspool.tile([128, COLS], f32, tag="p4", bufs=2, name=f"p4_{u}")
                    for m in range(4):
                        for k in range(4):
                            nc.tensor.matmul(
                                p4[:, m * BSH:(m + 1) * BSH], lhsT=wchunk(3, k, m),
                                rhs=act[:, k * BSH:(k + 1) * BSH],
                                start=(k == 0), stop=(k == 3))

                    v1 = hpool.tile([128, COLS], f32, tag="v1", name=f"v1_{u}")
                    nc.vector.scalar_tensor_tensor(
                        out=v1[:], in0=p4[:], scalar=cB[:, ds(it + u, 1)],
                        in1=ntiles[u][:], op0=mult, op1=add)
                    nc.vector.scalar_tensor_tensor(
                        out=x32n[:], in0=x32c[:], scalar=cA[:, ds(it + u, 1)],
                        in1=v1[:], op0=mult, op1=add)
                    nc.vector.tensor_copy(x16n[:], x32n[:])

            nc.sync.dma_start(xout_d[:], x32a[:])

    nc.compile()
    return nc


def get_nc(nsteps=T, unroll=UNROLL):
    key = (nsteps, unroll)
    if key not in _nc_cache:
        _nc_cache[key] = build_nc(nsteps, unroll)
    return _nc_cache[key]


# ---------------------------------------------------------------- entry point
def make_in_maps(inputs, nsteps=T):
    """Host-side prep: tables, weight packing, per-core noise shards."""
    x_init = np.asarray(inputs["x_init"], dtype=np.float32)
    step_noise = np.asarray(inputs["step_noise"], dtype=np.float32)
    w1 = np.asarray(inputs["w1"], dtype=np.float32)
    b1 = np.asarray(inputs["b1"], dtype=np.float32)
    w2 = np.asarray(inputs["w2"], dtype=np.float32)
    b2 = np.asarray(inputs["b2"], dtype=np.float32)
    w3 = np.asarray(inputs["w3"], dtype=np.float32)
    b3 = np.asarray(inputs["b3"], dtype=np.float32)
    w4 = np.asarray(inputs["w4"], dtype=np.float32)
    b4 = np.asarray(inputs["b4"], dtype=np.float32)
    tw1 = np.asarray(inputs["tw1"], dtype=np.float32)
    tb1 = np.asarray(inputs["tb1"], dtype=np.float32)
    tw2 = np.asarray(inputs["tw2"], dtype=np.float32)
    tb2 = np.asarray(inputs["tb2"], dtype=np.float32)

    A, Bc, C, bias1, ts_rev = host_tables(w1, b1, tw1, tb1, tw2, tb2, b4, nsteps)

    wts = pack_weights(w1, w2, w3, w4)

    # bias1 table: [128, 4*nsteps], col s*4 + c holds bias1[s, c*128+p]
    b1t = np.ascontiguousarray(
        bias1.reshape(nsteps, NCH, 128).transpose(2, 0, 1).reshape(128, nsteps * NCH)
    )
    b23 = np.zeros((128, 8), np.float32)
    b23[:, 0:4] = b2.reshape(NCH, 128).T
    b23[:, 4:8] = b3.reshape(NCH, 128).T
    cA_t = np.ascontiguousarray(np.broadcast_to(A[None, :], (128, nsteps)))
    cB_t = np.ascontiguousarray(np.broadcast_to(Bc[None, :], (128, nsteps)))

    # noise: the reference's scan pairs ts (reversed) with step_noise
    # POSITIONALLY — step s uses step_noise[s] (forward), t = T-1-s.
    # dev[s, p, c*16+b] = C[s]*noise[s, 16*core+b, c*128+p] + B[s]*b4[c*128+p]
    nrev = step_noise[:nsteps]                             # [S, 128, 512]
    b4fold = Bc[:, None] * b4[None, :]                     # [S, 512]
    in_maps = []
    for core in range(NCORES):
        v = nrev[:, BSH * core:BSH * (core + 1), :] * C[:, None, None] + b4fold[:, None, :]
        v = v.reshape(nsteps, BSH, NCH, 128).transpose(0, 3, 2, 1).reshape(nsteps * 128, COLS)
        in_maps.append({
            "wts": wts,
            "bias1": b1t,
            "b23": b23,
            "cA": cA_t,
            "cB": cB_t,
            "noise": np.ascontiguousarray(v.astype(np.float32)),
            "xinit": to_dev_layout(x_init[BSH * core:BSH * (core + 1)]),
        })
    return in_maps


def kernel(**inputs):
    global LAST_RESULTS
    from concourse.bass_utils import run_bass_kernel_spmd

    nc = get_nc(T, UNROLL)
    in_maps = make_in_maps(inputs, T)
    trace = os.environ.get("DIFF_TRACE", "0") == "1"
    res = run_bass_kernel_spmd(
        nc, in_maps, core_ids=list(range(NCORES)), trace=trace,
    )
    LAST_RESULTS = res
    out = np.concatenate(
        [from_dev_layout(r["xout"]) for r in res.results], axis=0
    ).astype(np.float32)
    return out


# revision 22
# speedup vs baseline: 956.5259x; 956.5259x over previous
"""Trainium2 Bass kernel for nn_DiffusionModel (reverse-diffusion sampling).

Strategy (data-parallel, per sharding hint):
  - 8 cores, batch 128 -> 16 rows per core; no cross-core communication.
  - 1000 serial steps; per step a 4-layer 512x512 MLP + elementwise update.
  - Weights live in SBUF in fp16 (PE fast-weight-load eligible), stationary
    operand of every matmul; activations kept feature-on-partition ("xT"
    layout, [128 partitions, 4 chunks x 16 batch cols]) so no transposes are
    ever needed: each layer's PSUM output is already next layer's input layout.
  - Time embedding is a per-step scalar -> folded into a per-step layer-1 bias
    table on the host (like betas/alphas_cumprod schedule).
  - Reverse-diffusion update x' = A_t*x + B_t*noise_pred + C_t*noise with
    host-precomputed A/B/C; C_t*noise (+ B_t*b4 fold) pre-scaled on host.
  - fp32 x master + fp32 PSUM accumulation; only the matmul datapath is fp16
    (validated: ~3e-5 relative error vs fp32 reference over 1000 steps).
"""

import os
import numpy as np

T = 1000
B = 128
D = 512
H = 512
BETA_START = 0.0001
BETA_END = 0.02
NCORES = 8
BSH = B // NCORES        # 16 batch rows per core
NCH = D // 128           # 4 feature chunks
COLS = NCH * BSH         # 64 sbuf cols per activation tile
UNROLL = 8

_nc_cache = {}
LAST_RESULTS = None


# ---------------------------------------------------------------- host tables
def host_tables(w1, b1, tw1, tb1, tw2, tb2, b4, nsteps=T):
    """Per-step (s = 0..T-1, t = T-1-s) coefficients and layer-1 bias table.

    x_{s+1} = A[s]*x + B[s]*noise_pred + C[s]*noise[t]
    bias1[s] = b1 + t_emb(t) * w1[512, :]
    """
    betas = np.linspace(BETA_START, BETA_END, T, dtype=np.float32)
    alphas = (1.0 - betas).astype(np.float32)
    ac = np.cumprod(alphas, dtype=np.float32)

    ts_rev = np.arange(T - 1, -1, -1)  # t for step s
    ac_t = ac[ts_rev].astype(np.float64)
    ac_prev = np.where(ts_rev > 0, ac[np.maximum(ts_rev - 1, 0)], 1.0).astype(np.float64)
    A = np.sqrt(ac_prev) / np.sqrt(ac_t)
    Bc = np.sqrt(1.0 - ac_prev) - A * np.sqrt(1.0 - ac_t)
    C = np.where(ts_rev > 0, np.sqrt(betas[ts_rev].astype(np.float64)), 0.0)

    # time-embedding scalar per t (same for every batch row)
    tnorm = (ts_rev.astype(np.float64) / T)[:, None]                  # [T,1]
    temb = np.maximum(tnorm @ tw1.astype(np.float64) + tb1.astype(np.float64), 0.0)
    temb = temb @ tw2.astype(np.float64) + tb2.astype(np.float64)     # [T,1]
    bias1 = b1.astype(np.float64)[None, :] + temb * w1[D].astype(np.float64)[None, :]

    A = A.astype(np.float32)[:nsteps]
    Bc = Bc.astype(np.float32)[:nsteps]
    C = C.astype(np.float32)[:nsteps]
    bias1 = bias1.astype(np.float32)[:nsteps]
    return A, Bc, C, bias1, ts_rev[:nsteps]


def pack_weights(w1, w2, w3, w4, np_wdt=np.float16):
    """[128, 4*16*128] fp16: layer l chunk (k,m) at cols ((l*16)+(k*4+m))*128."""
    out = []
    for w in (w1[:D], w2, w3, w4):
        r = w.reshape(NCH, 128, NCH, 128).transpose(1, 0, 2, 3).reshape(128, NCH * NCH * 128)
        out.append(r)
    return np.ascontiguousarray(np.concatenate(out, axis=1)).astype(np_wdt)


def to_dev_layout(x):
    """[16, 512] -> [128, 64] with dev[p, c*16+b] = x[b, c*128+p]."""
    return np.ascontiguousarray(
        x.reshape(BSH, NCH, 128).transpose(2, 1, 0).reshape(128, COLS)
    )


def from_dev_layout(xd):
    """[128, 64] -> [16, 512]."""
    return np.ascontiguousarray(
        xd.reshape(128, NCH, BSH).transpose(2, 1, 0).reshape(BSH, D)
    )


# ---------------------------------------------------------------- bass kernel
def build_nc(nsteps=T, unroll=UNROLL, run_steps=None, repeat=1,
             staggered=False, wdt="float16", probe_layers=3,
             probe_static_dma=False, fast_relu=True):
    """run_steps: iterate only this many steps (same tensor shapes) — used to
    isolate device exec time by wall-clock differencing.
    repeat: wrap the step loop in an outer For_i running it `repeat` times
    (timing probes only — state just keeps evolving)."""
    import concourse.bass as bass
    import concourse.mybir as mybir
    import concourse.tile as tile
    from concourse import bacc
    from concourse.bass import ds

    f32 = mybir.dt.float32
    f16 = mybir.dt.float16
    fw = getattr(mybir.dt, wdt)
    Relu = mybir.ActivationFunctionType.Relu
    add = mybir.AluOpType.add
    mult = mybir.AluOpType.mult
    amax = mybir.AluOpType.max

    if run_steps is None:
        run_steps = nsteps
    assert nsteps % unroll == 0 and run_steps % unroll == 0

    nc = bacc.Bacc("TRN2", target_bir_lowering=False)

    wts_d = nc.dram_tensor("wts", [128, 4 * 16 * 128], fw, kind="ExternalInput")
    bias1_d = nc.dram_tensor("bias1", [128, 4 * nsteps], f32, kind="ExternalInput")
    b23_d = nc.dram_tensor("b23", [128, 8], f32, kind="ExternalInput")
    cA_d = nc.dram_tensor("cA", [128, nsteps], f32, kind="ExternalInput")
    cB_d = nc.dram_tensor("cB", [128, nsteps], f32, kind="ExternalInput")
    noise_d = nc.dram_tensor("noise", [nsteps * 128, COLS], f32, kind="ExternalInput")
    xinit_d = nc.dram_tensor("xinit", [128, COLS], f32, kind="ExternalInput")
    xout_d = nc.dram_tensor("xout", [128, COLS], f32, kind="ExternalOutput")

    with tile.TileContext(nc) as tc:
        with (
            tc.tile_pool(name="const", bufs=1) as cpool,
            tc.tile_pool(name="noise", bufs=unroll) as npool,
            tc.tile_pool(name="acts", bufs=2) as hpool,
            tc.tile_pool(name="ps", bufs=1, space="PSUM") as pspool,
        ):
            wts = cpool.tile([128, 4 * 16 * 128], fw, tag="wts")
            bias1 = cpool.tile([128, 4 * nsteps], f32, tag="bias1")
            b23 = cpool.tile([128, 8], f32, tag="b23")
            cA = cpool.tile([128, nsteps], f32, tag="cA")
            cB = cpool.tile([128, nsteps], f32, tag="cB")
            x32a = cpool.tile([128, COLS], f32, tag="x32a")
            x32b = cpool.tile([128, COLS], f32, tag="x32b")
            x16a = cpool.tile([128, COLS], f16, tag="x16a")
            x16b = cpool.tile([128, COLS], f16, tag="x16b")

            nc.sync.dma_start(wts[:], wts_d[:])
            nc.sync.dma_start(bias1[:], bias1_d[:])
            nc.sync.dma_start(b23[:], b23_d[:])
            nc.sync.dma_start(cA[:], cA_d[:])
            nc.sync.dma_start(cB[:], cB_d[:])
            nc.sync.dma_start(x32a[:], xinit_d[:])
            nc.vector.tensor_copy(x16a[:], x32a[:])

            def wchunk(l, k, m):
                c0 = (l * 16 + k * 4 + m) * 128
                return wts[:, c0:c0 + 128]

            cA_st = cpool.tile([128, unroll], f32, tag="cA_st")
            cB_st = cpool.tile([128, unroll], f32, tag="cB_st")
            b1_st = cpool.tile([128, 4 * unroll], f32, tag="b1_st")

            import contextlib
            outer = (tc.For_i(0, repeat, 1) if repeat > 1
                     else contextlib.nullcontext())
            with outer, tc.For_i(0, run_steps, unroll,
                                 hint_engines=(mybir.EngineType.PE,),
                                 staggered_reset=staggered) as it:
                # stage this iteration's per-step scalars with static layout
                # (gpsimd is otherwise idle; one dynamic-offset op per table
                # instead of one per consumer op)
                nc.gpsimd.tensor_copy(cA_st[:], cA[:, ds(it, unroll)])
                nc.gpsimd.tensor_copy(cB_st[:], cB[:, ds(it, unroll)])
                nc.gpsimd.tensor_copy(b1_st[:], bias1[:, ds(it * 4, 4 * unroll)])

                ntiles = []
                for u in range(unroll):
                    nt = npool.tile([128, COLS], f32, tag="noise", name=f"nt{u}")
                    if probe_static_dma:
                        nc.sync.dma_start(nt[:], noise_d[u * 128:(u + 1) * 128, :])
                    else:
                        nc.sync.dma_start(nt[:], noise_d[ds(it * 128 + u * 128, 128), :])
                    ntiles.append(nt)

                for u in range(unroll):
                    if u % 2 == 0:
                        x32c, x16c, x32n, x16n = x32a, x16a, x32b, x16b
                    else:
                        x32c, x16c, x32n, x16n = x32b, x16b, x32a, x16a

                    act = x16c
                    for l in range(probe_layers):
                        pmA = pspool.tile([128, BSH], f32, tag="pmA", bufs=2,
                                          name=f"pmA{u}_{l}")
                        pmB = pspool.tile([128, BSH], f32, tag="pmB", bufs=2,
                                          name=f"pmB{u}_{l}")
                        pmCD = pspool.tile([128, 2 * BSH], f32, tag="pmCD", bufs=2,
                                           name=f"pmCD{u}_{l}")
                        pms = [pmA[:, :], pmB[:, :], pmCD[:, 0:BSH], pmCD[:, BSH:2 * BSH]]
                        # k-outer issue order: the matmul consuming the
                        # latest-evacuated input chunk lands 12 slots in,
                        # hiding the previous layer's evac latency
                        for k in range(4):
                            for m in range(4):
                                if m < 2:
                                    st, sp = (k == 0), (k == 3)
                                else:
                                    # pmCD holds m2+m3 as ONE accumulation
                                    # group (same PSUM zero region)
                                    st = (k == 0 and m == 2)
                                    sp = (k == 3 and m == 3)
                                nc.tensor.matmul(
                                    pms[m], lhsT=wchunk(l, k, m),
                                    rhs=act[:, k * BSH:(k + 1) * BSH],
                                    start=st, stop=sp)
                        h = hpool.tile([128, COLS], f16, tag=f"h{l}", name=f"h{u}_{l}")
                        if l == 0:
                            for c in range(4):
                                bcol = b1_st[:, u * 4 + c:u * 4 + c + 1]
                                if c % 2 == 0:
                                    nc.scalar.activation(
                                        h[:, c * BSH:(c + 1) * BSH], pms[c],
                                        Relu, bias=bcol)
                                else:
                                    nc.vector.tensor_scalar(
                                        out=h[:, c * BSH:(c + 1) * BSH],
                                        in0=pms[c], scalar1=bcol,
                                        scalar2=0.0, op0=add, op1=amax)
                        elif fast_relu:
                            nc.scalar.activation(h[:, 0:BSH], pms[0], Relu)
                            nc.vector.tensor_scalar(
                                out=h[:, BSH:2 * BSH], in0=pms[1], scalar1=0.0,
                                scalar2=None, op0=amax)
                            nc.scalar.activation(h[:, 2 * BSH:4 * BSH], pmCD[:, :],
                                                 Relu)
                        else:
                            for c in range(4):
                                bcol = b23[:, (l - 1) * 4 + c:(l - 1) * 4 + c + 1]
                                if c % 2 == 0:
                                    nc.scalar.activation(
                                        h[:, c * BSH:(c + 1) * BSH], pms[c],
                                        Relu, bias=bcol)
                                else:
                                    nc.vector.tensor_scalar(
                                        out=h[:, c * BSH:(c + 1) * BSH],
                                        in0=pms[c], scalar1=bcol,
                                        scalar2=0.0, op0=add, op1=amax)
                        act = h

                    p4 = pspool.tile([128, COLS], f32, tag="p4", bufs=2, name=f"p4_{u}")
                    for k in range(4):
                        for m in range(4):
                            nc.tensor.matmul(
                                p4[:, m * BSH:(m + 1) * BSH], lhsT=wchunk(3, k, m),
                                rhs=act[:, k * BSH:(k + 1) * BSH],
                                start=(k == 0 and m == 0),
                                stop=(k == 3 and m == 3))

                    v1 = hpool.tile([128, COLS], f32, tag="v1", name=f"v1_{u}")
                    nc.vector.scalar_tensor_tensor(
                        out=v1[:], in0=p4[:], scalar=cB_st[:, u:u + 1],
                        in1=ntiles[u][:], op0=mult, op1=add)
                    # critical path: fp16 x for next step's matmuls
                    nc.vector.scalar_tensor_tensor(
                        out=x16n[:], in0=x32c[:], scalar=cA_st[:, u:u + 1],
                        in1=v1[:], op0=mult, op1=add)
                    # off-path: fp32 master for the next tail
                    nc.vector.scalar_tensor_tensor(
                        out=x32n[:], in0=x32c[:], scalar=cA_st[:, u:u + 1],
                        in1=v1[:], op0=mult, op1=add)

            nc.sync.dma_start(xout_d[:], x32a[:])

    nc.compile()
    return nc


def get_nc(nsteps=T, unroll=UNROLL, run_steps=None, repeat=1, staggered=False,
           wdt="float16", probe_layers=3, probe_static_dma=False,
           fast_relu=True):
    key = (nsteps, unroll, run_steps, repeat, staggered, wdt, probe_layers,
           probe_static_dma, fast_relu)
    if key not in _nc_cache:
        _nc_cache[key] = build_nc(nsteps, unroll, run_steps, repeat, staggered,
                                  wdt, probe_layers, probe_static_dma,
                                  fast_relu)
    return _nc_cache[key]


# ---------------------------------------------------------------- entry point
def make_in_maps(inputs, nsteps=T):
    """Host-side prep: tables, weight packing, per-core noise shards."""
    x_init = np.asarray(inputs["x_init"], dtype=np.float32)
    step_noise = np.asarray(inputs["step_noise"], dtype=np.float32)
    w1 = np.asarray(inputs["w1"], dtype=np.float32)
    b1 = np.asarray(inputs["b1"], dtype=np.float32)
    w2 = np.asarray(inputs["w2"], dtype=np.float32)
    b2 = np.asarray(inputs["b2"], dtype=np.float32)
    w3 = np.asarray(inputs["w3"], dtype=np.float32)
    b3 = np.asarray(inputs["b3"], dtype=np.float32)
    w4 = np.asarray(inputs["w4"], dtype=np.float32)
    b4 = np.asarray(inputs["b4"], dtype=np.float32)
    tw1 = np.asarray(inputs["tw1"], dtype=np.float32)
    tb1 = np.asarray(inputs["tb1"], dtype=np.float32)
    tw2 = np.asarray(inputs["tw2"], dtype=np.float32)
    tb2 = np.asarray(inputs["tb2"], dtype=np.float32)

    A, Bc, C, bias1, ts_rev = host_tables(w1, b1, tw1, tb1, tw2, tb2, b4, nsteps)

    wts = pack_weights(w1, w2, w3, w4)

    # bias1 table: [128, 4*nsteps], col s*4 + c holds bias1[s, c*128+p]
    b1t = np.ascontiguousarray(
        bias1.reshape(nsteps, NCH, 128).transpose(2, 0, 1).reshape(128, nsteps * NCH)
    )
    b23 = np.zeros((128, 8), np.float32)
    b23[:, 0:4] = b2.reshape(NCH, 128).T
    b23[:, 4:8] = b3.reshape(NCH, 128).T
    cA_t = np.ascontiguousarray(np.broadcast_to(A[None, :], (128, nsteps)))
    cB_t = np.ascontiguousarray(np.broadcast_to(Bc[None, :], (128, nsteps)))

    # noise: the reference's scan pairs ts (reversed) with step_noise
    # POSITIONALLY — step s uses step_noise[s] (forward), t = T-1-s.
    # dev[s, p, c*16+b] = C[s]*noise[s, 16*core+b, c*128+p] + B[s]*b4[c*128+p]
    nrev = step_noise[:nsteps]                             # [S, 128, 512]
    b4fold = Bc[:, None] * b4[None, :]                     # [S, 512]
    in_maps = []
    for core in range(NCORES):
        v = nrev[:, BSH * core:BSH * (core + 1), :] * C[:, None, None] + b4fold[:, None, :]
        v = v.reshape(nsteps, BSH, NCH, 128).transpose(0, 3, 2, 1).reshape(nsteps * 128, COLS)
        in_maps.append({
            "wts": wts,
            "bias1": b1t,
            "b23": b23,
            "cA": cA_t,
            "cB": cB_t,
            "noise": np.ascontiguousarray(v.astype(np.float32)),
            "xinit": to_dev_layout(x_init[BSH * core:BSH * (core + 1)]),
        })
    return in_maps


def kernel(**inputs):
    global LAST_RESULTS
    from concourse.bass_utils import run_bass_kernel_spmd

    fast = (not np.any(np.asarray(inputs["b2"]))
            and not np.any(np.asarray(inputs["b3"])))
    nc = get_nc(T, UNROLL, fast_relu=fast)
    in_maps = make_in_maps(inputs, T)
    trace = os.environ.get("DIFF_TRACE", "0") == "1"
    res = run_bass_kernel_spmd(
        nc, in_maps, core_ids=list(range(NCORES)), trace=trace,
    )
    LAST_RESULTS = res
    out = np.concatenate(
        [from_dev_layout(r["xout"]) for r in res.results], axis=0
    ).astype(np.float32)
    return out
